# Initial kernel scaffold
#
"""FHN spectral attention kernel for 8 TRN2 NeuronCores.

Data-parallel over B=8 (one batch element per core). The reference math is
reassociated exactly so the [T,D]@[D,3D] qkv matmul never happens:

    xs[k,d]      = sum_t basis[t,k] x[t,d]                  (contract T)
    qkv_spec     = xs @ w_qkv.T          [32, 2304]
    attn[k,h]    = sum_d q_spec*k_spec / sqrt(64) * sigmoid(filt)
    fhn          = FHN(attn)             [32, 12]
    out_spec     = fhn (bcast d) * v_spec                    [32, 768]
    final_spec   = out_spec @ w_out.T    [32, 768]
    y.T[e,t]     = sum_k final_spec[k,e] basis[t,k]          (expand T)

All matmuls run in float32r (fp32 data, PE streams 1 row/cycle instead of
fp32's 4 -- tf32-class precision, measured 3.7e-4 of output absmax against
the fp32 reference). Weights/basisT are pre-transposed on the host so no
weight transposes happen on device; the two T-streaming matmuls (xs reduce,
y expand) run at the per-core HBM roofline (~390 GB/s measured), and the
spectral-domain middle phase overlaps into the stream tails.

Measured on 8 axon-tunneled trn2 cores: ~115-138 us whole-NEFF exec
(36 MB/core of HBM traffic; chip-level memory roofline ~100 us).
"""

import numpy as np

import concourse.bass as bass
import concourse.mybir as mybir
from concourse import bacc
import concourse.tile as tile
from concourse.bass_utils import run_bass_kernel_spmd
from concourse.masks import make_identity

F32 = mybir.dt.float32
F32R = mybir.dt.float32r
F16 = mybir.dt.float16

T, D = 4096, 768
H, HD, K = 12, 64, 32
D2 = 2 * D          # q,k columns
N_CORES = 8

TCH = 128           # t rows per matmul chunk
NT = T // TCH       # 32
XB = 4              # t-chunks per x DMA (512 rows, 1.5MB)
NQ = 1              # T quarters
QT = NT // NQ       # 8 t-chunks per quarter
DCH = 128           # d per chunk
ND = D // DCH       # 6

TAU, THRESH = 12.5, 0.5
A_PARAM, B_PARAM, DT = 0.7, 0.8, 1.0
ALPHA = DT / TAU
INV_DENOM = 1.0 / (1.0 + ALPHA * B_PARAM)


def _mm_slices(total, step=512):
    out = []
    s = 0
    while s < total:
        out.append((s, min(step, total - s)))
        s += step
    return out


def build_nc() -> bass.Bass:
    nc = bacc.Bacc(None, target_bir_lowering=False)

    x = nc.dram_tensor("x", [T, D], F16, kind="ExternalInput")
    basis = nc.dram_tensor("basis", [T, K], F16, kind="ExternalInput")
    basisT = nc.dram_tensor("basisT", [K, T], F32R, kind="ExternalInput")
    wqkvT = nc.dram_tensor("wqkvT", [D, D + D2], F16, kind="ExternalInput")
    woutT = nc.dram_tensor("woutT", [D, D], F32R, kind="ExternalInput")
    filtT = nc.dram_tensor("filtT", [K, H], F32, kind="ExternalInput")
    yT = nc.dram_tensor("yT", [D, T], F32, kind="ExternalOutput")

    with tile.TileContext(nc) as tc:
        _body(tc, x, basis, basisT, wqkvT, woutT, filtT, yT)
    nc.finalize()
    return nc


def _body(tc, x, basis, basisT, wqkvT, woutT, filtT, yT):
    nc = tc.nc

    with (
        tc.tile_pool(name="singles", bufs=1) as singles,
        tc.tile_pool(name="xin", bufs=3) as xin,
        tc.tile_pool(name="spec", bufs=1) as spec,
        tc.tile_pool(name="xsq", bufs=2) as xsq,
        tc.tile_pool(name="fhn", bufs=1) as fhn_pool,
        tc.tile_pool(name="yout", bufs=3) as yout,
    ):
        # ---- tiny early work: filter sigmoid, identity, constants ----------
        sb_filtT = singles.tile([K, H], F32)
        nc.sync.dma_start(sb_filtT, filtT[:, :])
        filt_sig = singles.tile([K, H], F32)
        nc.scalar.activation(filt_sig, sb_filtT, mybir.ActivationFunctionType.Sigmoid)
        neg5 = singles.tile([K, 1], F32)
        nc.vector.memset(neg5, -5.0)
        ident = singles.tile([K, K], F32)
        make_identity(nc, ident)

        sb_basis = singles.tile([TCH, NT, K], F16)
        bre = basis[:, :].rearrange("(n p) k -> p n k", p=TCH)
        for bq in range(4):
            nc.sync.dma_start(sb_basis[:, bq * 8:(bq + 1) * 8, :], bre[:, bq * 8:(bq + 1) * 8, :])

        sb_wqkvT = singles.tile([DCH, ND, D + D2], F16)
        sb_woutT = singles.tile([DCH, ND, D], F32R)
        sb_basisT = singles.tile([K, T], F32R)

        xre = x[:, :].rearrange("(n p) d -> p n d", p=TCH)

        out_spec = spec.tile([K, D], F32)
        qkv_acc = [spec.tile([K, D + D2], F32, name=f"qkvacc{i}", tag=f"qkvacc{i}") for i in range(2)]

        with (
            tc.tile_pool(name="psA", bufs=1, space="PSUM") as psA,
            tc.tile_pool(name="psT", bufs=1, space="PSUM") as psT,
            tc.tile_pool(name="psQ", bufs=1, space="PSUM") as psQ,
        ):
            for q in range(NQ):
                # ---- xs_q[k, d] = sum_{t in quarter} basis[t,k] x[t,d] ----
                ps_xs = psA.tile([K, D], F32, tag="ps_xs")
                # first transfer is 1 chunk (fast pipeline start), then a
                # 3-chunk catch-up, then full XB-sized groups
                groups = []
                pos = q * QT
                end = (q + 1) * QT
                if q == 0 and QT > XB:
                    groups += [(pos, 1), (pos + 1, XB - 1)]
                    pos += XB
                while pos < end:
                    n = min(XB, end - pos)
                    groups.append((pos, n))
                    pos += n
                for gi, n in groups:
                    x_tile = xin.tile([TCH, XB, D], F16, name="x_tile", tag="x_tile")
                    nc.sync.dma_start(x_tile[:, :n, :], xre[:, gi:gi + n, :])
                    for j in range(n):
                        i = gi + j
                        for (s, w) in _mm_slices(D):
                            nc.tensor.matmul(
                                ps_xs[:, s:s + w],
                                lhsT=sb_basis[:, i, :],
                                rhs=x_tile[:, j, s:s + w],
                                start=(i % QT == 0),
                                stop=(i % QT == QT - 1),
                            )
                # stream weights in behind the first quarters of x
                if q == 0:
                    for dc in range(ND):
                        nc.sync.dma_start(sb_wqkvT[:, dc, :], wqkvT[dc * DCH:(dc + 1) * DCH, :])
                if q == min(1, NQ - 1):
                    for dc in range(ND):
                        nc.sync.dma_start(sb_woutT[:, dc, :], woutT[dc * DCH:(dc + 1) * DCH, :])
                    nc.sync.dma_start(sb_basisT, basisT[:, :])

                sb_xs = xsq.tile([K, D], F32, tag="sb_xs")
                nc.vector.tensor_copy(sb_xs, ps_xs)

                # transpose xs_q -> 6 chunks [128, 32]
                xsT_f = spec.tile([DCH, ND, K], F16, tag=f"xsTf{q}")
                for dc in range(ND):
                    ps_t = psT.tile([DCH, K], F32, tag="ps_t")
                    nc.tensor.transpose(ps_t, sb_xs[:, dc * DCH:(dc + 1) * DCH], ident)
                    nc.vector.tensor_copy(xsT_f[:, dc, :], ps_t)

                # this quarter's q/k/v projection partial, folded into the
                # SBUF accumulator (short self-contained PSUM groups only)
                ps_qkv = psQ.tile([K, D + D2], F32, tag="ps_qkv")
                for dc in range(ND):
                    for (s, w) in _mm_slices(D + D2):
                        nc.tensor.matmul(
                            ps_qkv[:, s:s + w],
                            lhsT=xsT_f[:, dc, :],
                            rhs=sb_wqkvT[:, dc, s:s + w],
                            start=(dc == 0),
                            stop=(dc == ND - 1),
                        )
                if q == 0:
                    nc.vector.tensor_copy(qkv_acc[0], ps_qkv)
                else:
                    nc.vector.tensor_tensor(
                        qkv_acc[q % 2], qkv_acc[(q + 1) % 2], ps_qkv,
                        op=mybir.AluOpType.add,
                    )

            # ---- attention scalar + FHN on [K, H] --------------------------
            qkv = qkv_acc[(NQ - 1) % 2]
            fp = fhn_pool
            prod = fp.tile([K, D], F32)
            nc.vector.tensor_mul(prod, qkv[:, 0:D], qkv[:, D:D2])
            red = fp.tile([K, H], F32)
            nc.vector.reduce_sum(
                red, prod.rearrange("p (h d) -> p h d", d=HD), axis=mybir.AxisListType.X
            )
            stim = fp.tile([K, H], F32)
            nc.vector.scalar_tensor_tensor(
                stim, red, 1.0 / (HD ** 0.5), filt_sig,
                op0=mybir.AluOpType.mult, op1=mybir.AluOpType.mult,
            )
            # scale = max(|stim|, 1e-6) = max(max(stim, -stim), 1e-6)
            ab = fp.tile([K, H], F32)
            nc.vector.scalar_tensor_tensor(
                ab, stim, -1.0, stim, op0=mybir.AluOpType.mult, op1=mybir.AluOpType.max
            )
            scale = fp.tile([K, H], F32)
            nc.vector.tensor_scalar_max(scale, ab, 1e-6)
            rscale = fp.tile([K, H], F32)
            nc.vector.reciprocal(rscale, scale)
            gate = fp.tile([K, H], F32)
            nc.scalar.activation(
                gate, scale, mybir.ActivationFunctionType.Sigmoid, bias=neg5, scale=10.0
            )
            g9 = fp.tile([K, H], F32)
            nc.vector.tensor_scalar(
                g9, gate, 0.9, 0.1, op0=mybir.AluOpType.mult, op1=mybir.AluOpType.add
            )
            sn = fp.tile([K, H], F32)
            nc.vector.tensor_mul(sn, stim, rscale)
            v1 = fp.tile([K, H], F32)   # == I (first FHN step from v=w=0, |I|<=1)
            nc.vector.tensor_mul(v1, g9, sn)
            w1 = fp.tile([K, H], F32)   # (v1+A)*alpha/denom; clip never binds
            nc.vector.tensor_scalar(
                w1, v1, A_PARAM, ALPHA * INV_DENOM,
                op0=mybir.AluOpType.add, op1=mybir.AluOpType.mult,
            )
            # step 2: v2 = clip(3*v1 - v1^3/3 - w1, +-3)   (dv uses I == v1)
            c1 = fp.tile([K, H], F32)
            nc.vector.tensor_mul(c1, v1, v1)
            c2 = fp.tile([K, H], F32)
            nc.vector.tensor_mul(c2, c1, v1)
            u1 = fp.tile([K, H], F32)
            nc.vector.scalar_tensor_tensor(
                u1, c2, -1.0 / 3.0, w1, op0=mybir.AluOpType.mult, op1=mybir.AluOpType.subtract
            )  # -c/3 - w1
            u2 = fp.tile([K, H], F32)
            nc.vector.scalar_tensor_tensor(
                u2, v1, 3.0, u1, op0=mybir.AluOpType.mult, op1=mybir.AluOpType.add
            )  # 3*v1 - c/3 - w1
            v2 = fp.tile([K, H], F32)
            nc.vector.tensor_scalar(
                v2, u2, 3.0, -3.0, op0=mybir.AluOpType.min, op1=mybir.AluOpType.max
            )
            fhn = fp.tile([K, H], F32)
            nc.vector.tensor_mul(fhn, v2, scale)

            # ---- out_spec = fhn (bcast over d) * v_spec --------------------
            for h in range(H):
                nc.vector.tensor_scalar_mul(
                    out_spec[:, h * HD:(h + 1) * HD],
                    qkv[:, D2 + h * HD:D2 + (h + 1) * HD],
                    fhn[:, h:h + 1],
                )

        # ---- final_spec = out_spec @ wout.T (fp32r) ------------------------
        sb_fs = spec.tile([K, D], F32R)
        with (
            tc.tile_pool(name="psT2", bufs=2, space="PSUM") as psT2,
            tc.tile_pool(name="psF", bufs=1, space="PSUM") as psF,
        ):
            sb_osT = spec.tile([DCH, ND, K], F32R)
            for dc in range(ND):
                ps_t = psT2.tile([DCH, K], F32)
                nc.tensor.transpose(ps_t, out_spec[:, dc * DCH:(dc + 1) * DCH], ident)
                nc.vector.tensor_copy(sb_osT[:, dc, :], ps_t)
            ps_fs = psF.tile([K, D], F32)
            for dc in range(ND):
                for (s, w) in _mm_slices(D):
                    nc.tensor.matmul(
                        ps_fs[:, s:s + w],
                        lhsT=sb_osT[:, dc, :],
                        rhs=sb_woutT[:, dc, s:s + w],
                        start=(dc == 0),
                        stop=(dc == ND - 1),
                    )
            nc.vector.tensor_copy(sb_fs, ps_fs)

        # ---- yT[e, t] = sum_k final_spec[k, e] * basisT[k, t] (fp32r) ------
        # deep PSUM pipeline: matmuls stream ahead of the PSUM->SBUF copies,
        # copies alternate DVE/ACT, DMA out per half-row (1MB) for smoothness
        HT = T // 2
        with tc.tile_pool(name="psY", bufs=8, space="PSUM") as psY:
            for ec in range(ND):
                for half in range(2):
                    y_tile = yout.tile([DCH, HT], F32, name="y_tile", tag="y_tile")
                    for ti, (s, w) in enumerate(_mm_slices(HT)):
                        gs = half * HT + s
                        ps_y = psY.tile([DCH, 512], F32, tag="ps_y")
                        nc.tensor.matmul(
                            ps_y[:, :w],
                            lhsT=sb_fs[:, ec * DCH:(ec + 1) * DCH],
                            rhs=sb_basisT[:, gs:gs + w],
                            start=True,
                            stop=True,
                        )
                        if ti % 2 == 0:
                            nc.vector.tensor_copy(y_tile[:, s:s + w], ps_y[:, :w])
                        else:
                            nc.scalar.copy(y_tile[:, s:s + w], ps_y[:, :w])
                    nc.sync.dma_start(
                        yT[ec * DCH:(ec + 1) * DCH, half * HT:(half + 1) * HT], y_tile
                    )


_NC_CACHE = None


def _get_nc():
    global _NC_CACHE
    if _NC_CACHE is None:
        _NC_CACHE = build_nc()
    return _NC_CACHE


def _prep_in_maps(x, spectral_basis, w_qkv, w_out, spectral_filter):
    x = np.asarray(x, dtype=np.float16)
    spectral_basis = np.asarray(spectral_basis, dtype=np.float32)
    basis16 = spectral_basis.astype(np.float16)
    wqkvT = np.ascontiguousarray(np.asarray(w_qkv, dtype=np.float32).T.astype(np.float16))
    woutT = np.ascontiguousarray(np.asarray(w_out, dtype=np.float32).T)
    filtT = np.ascontiguousarray(np.asarray(spectral_filter, dtype=np.float32).T[:K, :])
    in_maps = []
    for c in range(N_CORES):
        in_maps.append({
            "x": np.ascontiguousarray(x[c]),
            "basis": np.ascontiguousarray(basis16[c]),
            "basisT": np.ascontiguousarray(spectral_basis[c].T),
            "wqkvT": wqkvT,
            "woutT": woutT,
            "filtT": filtT,
        })
    return in_maps


def kernel(x, spectral_basis, w_qkv, w_out, spectral_filter):
    in_maps = _prep_in_maps(x, spectral_basis, w_qkv, w_out, spectral_filter)
    last_err = None
    for attempt in range(3):
        try:
            res = run_bass_kernel_spmd(_get_nc(), in_maps, core_ids=list(range(N_CORES)))
            break
        except Exception as e:  # transient NRT device errors recover on retry
            last_err = e
            import time
            time.sleep(2.0 * (attempt + 1))
    else:
        raise last_err
    out = np.stack([res.results[c]["yT"].T for c in range(N_CORES)])
    return np.ascontiguousarray(out.astype(np.float32))


def kernel_profiled(x, spectral_basis, w_qkv, w_out, spectral_filter, tmpdir=None):
    """Same as kernel() but with NTFF tracing; returns (out, BassKernelResults)."""
    in_maps = _prep_in_maps(x, spectral_basis, w_qkv, w_out, spectral_filter)
    res = run_bass_kernel_spmd(
        _get_nc(), in_maps, core_ids=list(range(N_CORES)),
        trace=True, trace_cores=list(range(N_CORES)), tmpdir=tmpdir,
    )
    out = np.stack([res.results[c]["yT"].T for c in range(N_CORES)])
    return np.ascontiguousarray(out.astype(np.float32)), res



# revision 22
# speedup vs baseline: 1.1804x; 1.1804x over previous
"""FHN spectral attention kernel for 8 TRN2 NeuronCores.

Data-parallel over B=8 (one batch element per core). The reference math is
reassociated exactly so the [T,D]@[D,3D] qkv matmul never happens:

    xs[k,d]      = sum_t basis[t,k] x[t,d]                  (contract T)
    qkv_spec     = xs @ w_qkv.T          [32, 2304]
    attn[k,h]    = sum_d q_spec*k_spec / sqrt(64) * sigmoid(filt)
    fhn          = FHN(attn)             [32, 12]
    out_spec     = fhn (bcast d) * v_spec                    [32, 768]
    final_spec   = out_spec @ w_out.T    [32, 768]
    y.T[e,t]     = sum_k final_spec[k,e] basis[t,k]          (expand T)

All HBM-crossing tensors are f16 (pass gate is 2e-2 of absmax; measured error
~1e-3). y overflows f16 range, so 2^-7 is folded into the tiny fhn tile on
device and the host multiplies the output by 128 (exact).

Performance structure:
- x is host-packed so each partition's DMA line is one contiguous run.
- xs reduction col-packs 4 t-chunks into the PE array's 4 column groups
  (concurrent matmuls); the 4 partial sums are reduced + transposed in one
  matmul against a stacked identity.
- qkv weights are split q|k (streamed first) vs v (streamed last): the whole
  attention-scalar + FHN vector chain runs while the v/wout weights are still
  streaming in, taking it off the critical path.
- fhn is expanded to [d, k] layout by two tiny matmuls (a transpose against a
  repeated identity, then a head-selector matmul), so out_spec is built
  directly in the transposed layout the w_out projection needs -- no
  serial transpose pass after FHN.
- the y expansion row-packs the 4 t-quarters into the PE's 4 row groups
  (basisT loaded quarter-packed, final_spec replicated via the repeated
  identity), with [128,1024] PSUM tiles so PSUM->SBUF copies amortize.
- junk warm-up matmuls keep the PE HAM clock-gate at 8/8 through the DMA
  phases so the middle/expand matmuls run at 2.4 GHz, not 1.2.
"""

import numpy as np

import concourse.bass as bass
import concourse.mybir as mybir
from concourse import bacc
import concourse.tile as tile
from concourse.bass_utils import run_bass_kernel_spmd
from concourse.masks import make_identity

F32 = mybir.dt.float32
F16 = mybir.dt.float16

T, D = 4096, 768
H, HD, K = 12, 64, 32
D2 = 2 * D
D3 = 3 * D
QKW = 2 * D         # q|k weight columns
N_CORES = 8

TCH = 128           # t rows per matmul chunk
NT = T // TCH       # 32
XB = 4              # t-chunks per x DMA == col-pack width
DCH = 128           # d per chunk
ND = D // DCH       # 6
TQ = T // 4         # 1024, t per row-pack quarter

TAU, THRESH = 12.5, 0.5
A_PARAM, B_PARAM, DT = 0.7, 0.8, 1.0
ALPHA = DT / TAU
INV_DENOM = 1.0 / (1.0 + ALPHA * B_PARAM)
WC = ALPHA * INV_DENOM
OSHIFT = 2.0 ** -7  # fold into fhn so out_spec/final_spec/y all fit f16


def _mm_slices(total, step=512):
    out = []
    s = 0
    while s < total:
        out.append((s, min(step, total - s)))
        s += step
    return out


def build_nc() -> bass.Bass:
    nc = bacc.Bacc(None, target_bir_lowering=False)

    xH = nc.dram_tensor("xH", [TCH, NT * D], F16, kind="ExternalInput")
    basis = nc.dram_tensor("basis", [T, K], F16, kind="ExternalInput")
    basisTp = nc.dram_tensor("basisTp", [TCH, TQ], F16, kind="ExternalInput")
    wqkT = nc.dram_tensor("wqkT", [D, QKW], F16, kind="ExternalInput")
    wvT = nc.dram_tensor("wvT", [D, D], F16, kind="ExternalInput")
    woutT = nc.dram_tensor("woutT", [D, D], F16, kind="ExternalInput")
    filtT = nc.dram_tensor("filtT", [K, H], F32, kind="ExternalInput")
    yT = nc.dram_tensor("yT", [D, T], F16, kind="ExternalOutput")

    with tile.TileContext(nc) as tc:
        _body(tc, xH, basis, basisTp, wqkT, wvT, woutT, filtT, yT)
    nc.finalize()
    return nc


def _body(tc, xH, basis, basisTp, wqkT, wvT, woutT, filtT, yT):
    nc = tc.nc
    ActF = mybir.ActivationFunctionType
    Alu = mybir.AluOpType

    with (
        tc.tile_pool(name="singles", bufs=1) as singles,
        tc.tile_pool(name="xin", bufs=4) as xin,
        tc.tile_pool(name="spec", bufs=1) as spec,
        tc.tile_pool(name="fhn", bufs=1) as fp,
        tc.tile_pool(name="yout", bufs=3) as yout,
    ):
        # ---- constants (no DMA; engines are idle here anyway) --------------
        neg5 = singles.tile([K, 1], F32)
        nc.vector.memset(neg5, -5.0)
        # ident4[32g+k, k'] = d(k,k'): reduces 4 col-group partials via matmul
        ident4 = singles.tile([TCH, K], F16)
        nc.gpsimd.memset(ident4, 0.0)
        for g in range(4):
            make_identity(nc, ident4[32 * g:32 * (g + 1), :], nomemset=True)
        # ident_rep[k, 32g+k'] = d(k,k'): transpose that also replicates x4
        ident_rep = singles.tile([K, TCH], F16)
        nc.gpsimd.memset(ident_rep, 0.0)
        for g in range(4):
            make_identity(nc, ident_rep[:, 32 * g:32 * (g + 1)], nomemset=True)
        # S_all[h, d] = 1 iff h == d//64: head-selector for the fhn expansion
        S_all = singles.tile([H, D], F16)
        nc.gpsimd.memset(S_all, 0.0)
        sview = S_all.rearrange("h (d1 e) -> h d1 e", e=HD)
        nc.gpsimd.affine_select(
            out=sview, in_=sview, compare_op=mybir.AluOpType.not_equal,
            fill=1.0, base=0, pattern=[[-1, H], [0, HD]], channel_multiplier=1,
        )

        sb_filtT = singles.tile([K, H], F32)
        filt_sig = singles.tile([K, H], F32)
        sb_basis = singles.tile([TCH, NT, K], F16)
        bre = basis[:, :].rearrange("(n p) k -> p n k", p=TCH)

        sb_wqkT = singles.tile([DCH, ND, QKW], F16)
        sb_wvT = singles.tile([DCH, ND, D], F16)
        sb_woutT = singles.tile([DCH, ND, D], F16)
        sb_basisTp = singles.tile([TCH, TQ], F16)

        # ---- x phase: xs4[32g+k, d] = sum_{i%4==g} basis[chunk i] @ x ------
        xsT_f = spec.tile([DCH, ND, K], F16)
        with (
            tc.tile_pool(name="psA", bufs=1, space="PSUM") as psA,
            tc.tile_pool(name="psT", bufs=2, space="PSUM") as psT,
            tc.tile_pool(name="psJ", bufs=1, space="PSUM") as psJ,
        ):
            ps_xs4 = psA.tile([TCH, D], F32, tag="ps_xs4")
            groups = [(0, 1), (1, 3)]
            pos = XB
            while pos < NT:
                groups.append((pos, min(XB, NT - pos)))
                pos += XB
            nc.sync.dma_start(sb_basis[:, 0:8, :], bre[:, 0:8, :])
            for gidx, (gi, n) in enumerate(groups):
                x_tile = xin.tile([TCH, XB, D], F16, name="x_tile", tag="x_tile")
                src = xH[:, gi * D:(gi + n) * D].rearrange("p (n d) -> p n d", d=D)
                nc.sync.dma_start(x_tile[:, :n, :], src)
                if gidx == 0:
                    for bq in range(1, 4):
                        nc.sync.dma_start(sb_basis[:, bq * 8:(bq + 1) * 8, :],
                                          bre[:, bq * 8:(bq + 1) * 8, :])
                    nc.sync.dma_start(sb_filtT, filtT[:, :])
                    nc.scalar.activation(filt_sig, sb_filtT, ActF.Sigmoid)
                for j in range(n):
                    i = gi + j
                    g = i % 4
                    for (s, w) in _mm_slices(D):
                        nc.tensor.matmul(
                            ps_xs4[32 * g:32 * (g + 1), s:s + w],
                            lhsT=sb_basis[:, i, :],
                            rhs=x_tile[:, j, s:s + w],
                            start=(i < 4),
                            stop=(i >= NT - 4),
                            tile_position=(0, 32 * g),
                            skip_group_check=True,
                        )
                # HAM warm-up: keep the PE busy through the DMA wait so the
                # clock gate opens early and stays open (results unused)
                for _ in range(2):
                    ps_j = psJ.tile([K, 512], F32, tag="ps_j")
                    nc.tensor.matmul(
                        ps_j, lhsT=sb_basis[:, gi, :], rhs=x_tile[:, 0, 0:512],
                        start=True, stop=True,
                    )
            # weights stream behind the x DMAs, in consumption order.
            # wqkT is split 12 ways so the per-queue round-robin gives it
            # most of the bandwidth: it is consumed first but would otherwise
            # finish LAST (fair sharing drains small late transfers first).
            for dc in range(ND):
                for hf in range(2):
                    nc.sync.dma_start(
                        sb_wqkT[:, dc, hf * D:(hf + 1) * D],
                        wqkT[dc * DCH:(dc + 1) * DCH, hf * D:(hf + 1) * D])
            wvre = wvT[:, :].rearrange("(n p) d -> p n d", p=DCH)
            wore = woutT[:, :].rearrange("(n p) d -> p n d", p=DCH)
            nc.sync.dma_start(sb_wvT[:, 0:3, :], wvre[:, 0:3, :])
            nc.sync.dma_start(sb_wvT[:, 3:6, :], wvre[:, 3:6, :])
            nc.sync.dma_start(sb_woutT[:, 0:3, :], wore[:, 0:3, :])
            nc.sync.dma_start(sb_woutT[:, 3:6, :], wore[:, 3:6, :])
            nc.sync.dma_start(sb_basisTp, basisTp[:, :])

            # xs partials -> f16 SBUF; matmul vs stacked identity then does
            # the 4-way partial reduce AND the transpose in one shot
            sb_xs4 = spec.tile([TCH, D], F16)
            nc.vector.tensor_copy(sb_xs4[:, 0:D // 2], ps_xs4[:, 0:D // 2])
            nc.scalar.copy(sb_xs4[:, D // 2:D], ps_xs4[:, D // 2:D])
            for dc in range(ND):
                ps_t = psT.tile([DCH, K], F32, tag="ps_t")
                nc.tensor.matmul(
                    ps_t, lhsT=sb_xs4[:, dc * DCH:(dc + 1) * DCH], rhs=ident4,
                    start=True, stop=True,
                )
                if dc % 2 == 0:
                    nc.vector.tensor_copy(xsT_f[:, dc, :], ps_t)
                else:
                    nc.scalar.copy(xsT_f[:, dc, :], ps_t)

        # ---- spectral middle ------------------------------------------------
        sb_v = spec.tile([K, D], F16)
        vT4 = spec.tile([DCH, ND, TCH], F16)
        sb_osT4 = spec.tile([DCH, ND, TCH], F16)
        sb_fs4 = spec.tile([TCH, D], F16)
        with (
            tc.tile_pool(name="psQK", bufs=1, space="PSUM") as psQK,
            tc.tile_pool(name="psV", bufs=1, space="PSUM") as psV,
        ):
            ps_qk = psQK.tile([K, QKW], F32, tag="ps_qk")
            for dc in range(ND):
                for (s, w) in _mm_slices(QKW):
                    nc.tensor.matmul(
                        ps_qk[:, s:s + w],
                        lhsT=xsT_f[:, dc, :], rhs=sb_wqkT[:, dc, s:s + w],
                        start=(dc == 0), stop=(dc == ND - 1),
                    )
            # HAM bridge: keep the PE busy across the FHN window (which is
            # otherwise long enough for the clock gate to re-throttle) so
            # the expansion / w_out / y matmuls run at 2.4 GHz, not 1.2
            with tc.tile_pool(name="psJ2", bufs=1, space="PSUM") as psJ2:
                for _ in range(8):
                    ps_j2 = psJ2.tile([K, 512], F32, tag="ps_j2")
                    nc.tensor.matmul(
                        ps_j2, lhsT=xsT_f[:, 0, :], rhs=sb_wqkT[:, 0, 0:512],
                        start=True, stop=True,
                    )
            ps_v = psV.tile([K, D], F32, tag="ps_v")
            for dc in range(ND):
                for (s, w) in _mm_slices(D):
                    nc.tensor.matmul(
                        ps_v[:, s:s + w],
                        lhsT=xsT_f[:, dc, :], rhs=sb_wvT[:, dc, s:s + w],
                        start=(dc == 0), stop=(dc == ND - 1),
                    )

            # attention scalar: attn[k,h] = sum_d q*k / 8 * sigmoid(filt)
            sb_q = fp.tile([K, D], F32)
            nc.vector.tensor_copy(sb_q[:, 0:D // 2], ps_qk[:, 0:D // 2])
            nc.scalar.copy(sb_q[:, D // 2:D], ps_qk[:, D // 2:D])
            prod = fp.tile([K, D], F32)
            nc.vector.tensor_tensor(prod, sb_q, ps_qk[:, D:QKW], op=Alu.mult)
            red = fp.tile([K, H], F32)
            nc.vector.reduce_sum(
                red, prod.rearrange("p (h d) -> p h d", d=HD), axis=mybir.AxisListType.X
            )
            # FHN, DVE chain with the sigmoid/affine pieces on ACT in parallel
            stim = fp.tile([K, H], F32)
            nc.vector.scalar_tensor_tensor(
                stim, red, 1.0 / (HD ** 0.5), filt_sig, op0=Alu.mult, op1=Alu.mult
            )
            ab = fp.tile([K, H], F32)
            nc.vector.scalar_tensor_tensor(
                ab, stim, -1.0, stim, op0=Alu.mult, op1=Alu.max
            )
            scale = fp.tile([K, H], F32)
            nc.vector.tensor_scalar_max(scale, ab, 1e-6)
            gate = fp.tile([K, H], F32)
            nc.scalar.activation(gate, scale, ActF.Sigmoid, bias=neg5, scale=10.0)
            g9 = fp.tile([K, H], F32)
            nc.scalar.activation(g9, gate, ActF.Copy, bias=0.1, scale=0.9)
            rscale = fp.tile([K, H], F32)
            nc.vector.reciprocal(rscale, scale)
            sn = fp.tile([K, H], F32)
            nc.vector.tensor_mul(sn, stim, rscale)
            v1 = fp.tile([K, H], F32)   # == I (first FHN step from v=w=0, |I|<=1)
            nc.vector.tensor_mul(v1, sn, g9)
            w1 = fp.tile([K, H], F32)   # (v1+A)*alpha/denom; clip never binds
            nc.scalar.activation(w1, v1, ActF.Copy, bias=A_PARAM * WC, scale=WC)
            # v2 = clip(3*v1 - v1^3/3 - w1, +-3)   (dv uses I == v1)
            c1 = fp.tile([K, H], F32)
            nc.vector.tensor_mul(c1, v1, v1)
            c2 = fp.tile([K, H], F32)
            nc.vector.tensor_mul(c2, c1, v1)
            u1 = fp.tile([K, H], F32)
            nc.vector.scalar_tensor_tensor(
                u1, c2, -1.0 / 3.0, w1, op0=Alu.mult, op1=Alu.subtract
            )
            u2 = fp.tile([K, H], F32)
            nc.vector.scalar_tensor_tensor(
                u2, v1, 3.0, u1, op0=Alu.mult, op1=Alu.add
            )
            v2 = fp.tile([K, H], F32)
            nc.vector.tensor_scalar(v2, u2, 3.0, -3.0, op0=Alu.min, op1=Alu.max)
            fhn16 = fp.tile([K, H], F16)  # v2 * scale * 2^-7 (host undoes 2^-7)
            nc.vector.scalar_tensor_tensor(
                fhn16, v2, OSHIFT, scale, op0=Alu.mult, op1=Alu.mult
            )

            # v_spec to SBUF on ACT only, after the FHN ACT pieces, so the DVE
            # chain never waits on the (late) v weight stream
            nc.scalar.copy(sb_v[:, 0:D // 2], ps_v[:, 0:D // 2])
            nc.scalar.copy(sb_v[:, D // 2:D], ps_v[:, D // 2:D])

        with (
            tc.tile_pool(name="psT2", bufs=2, space="PSUM") as psT2,
            tc.tile_pool(name="psE", bufs=2, space="PSUM") as psE,
            tc.tile_pool(name="psF", bufs=1, space="PSUM") as psF,
        ):
            # vT4[d, 32g+k] = v_spec[k, d] replicated into all 4 row groups
            for dc in range(ND):
                ps_t2 = psT2.tile([DCH, TCH], F32, tag="ps_t2")
                nc.tensor.matmul(
                    ps_t2, lhsT=sb_v[:, dc * DCH:(dc + 1) * DCH], rhs=ident_rep,
                    start=True, stop=True,
                )
                nc.scalar.copy(vT4[:, dc, :], ps_t2)
            # fhnT_rep[h, 32g+k] = fhn[k, h]
            ps_fT = psT2.tile([H, TCH], F32, tag="ps_fT")
            nc.tensor.matmul(ps_fT, lhsT=fhn16, rhs=ident_rep, start=True, stop=True)
            fhnT_sb = fp.tile([H, TCH], F16)
            nc.vector.tensor_copy(fhnT_sb, ps_fT)
            # per d-chunk: E[d, 32g+k] = fhn[k, h(d)] via the head selector,
            # then out_specT = v_specT * E, then the w_out projection
            ps_fs4 = psF.tile([TCH, D], F32, tag="ps_fs4")
            for dc in range(ND):
                ps_e = psE.tile([DCH, TCH], F32, tag="ps_e")
                nc.tensor.matmul(
                    ps_e, lhsT=S_all[:, dc * DCH:(dc + 1) * DCH], rhs=fhnT_sb,
                    start=True, stop=True,
                )
                nc.vector.tensor_tensor(
                    sb_osT4[:, dc, :], vT4[:, dc, :], ps_e, op=Alu.mult
                )
                for (s, w) in _mm_slices(D):
                    nc.tensor.matmul(
                        ps_fs4[:, s:s + w],
                        lhsT=sb_osT4[:, dc, :], rhs=sb_woutT[:, dc, s:s + w],
                        start=(dc == 0), stop=(dc == ND - 1),
                    )
            nc.vector.tensor_copy(sb_fs4[:, 0:D // 2], ps_fs4[:, 0:D // 2])
            nc.scalar.copy(sb_fs4[:, D // 2:D], ps_fs4[:, D // 2:D])

        # ---- yT[e, t] = sum_k final_spec[k, e] * basisT[k, t] ---------------
        # row-packed 4x: row group g computes t-quarter g; [128,1024] PSUM
        # tiles so each PSUM->SBUF copy moves 2 banks; one 1MB DMA per e-chunk
        with tc.tile_pool(name="psY", bufs=4, space="PSUM") as psY:
            for ec in range(ND):
                y_tile = yout.tile([DCH, T], F16, name="y_tile", tag="y_tile")
                for g in range(4):
                    ps_y = psY.tile([DCH, TQ], F32, tag="ps_y")
                    for (s, w) in _mm_slices(TQ):
                        nc.tensor.matmul(
                            ps_y[:, s:s + w],
                            lhsT=sb_fs4[32 * g:32 * (g + 1), ec * DCH:(ec + 1) * DCH],
                            rhs=sb_basisTp[32 * g:32 * (g + 1), s:s + w],
                            start=True, stop=True,
                            tile_position=(32 * g, 0),
                        )
                    dst = y_tile[:, g * TQ:(g + 1) * TQ]
                    if (ec + g) % 2 == 0:
                        nc.vector.tensor_copy(dst, ps_y)
                    else:
                        nc.scalar.copy(dst, ps_y)
                nc.sync.dma_start(yT[ec * DCH:(ec + 1) * DCH, :], y_tile)


_NC_CACHE = None


def _get_nc():
    global _NC_CACHE
    if _NC_CACHE is None:
        _NC_CACHE = build_nc()
    return _NC_CACHE


def _prep_in_maps(x, spectral_basis, w_qkv, w_out, spectral_filter):
    x16 = np.asarray(x, dtype=np.float16)
    basis16 = np.asarray(spectral_basis, dtype=np.float32).astype(np.float16)
    wqkvT = np.asarray(w_qkv, dtype=np.float32).T.astype(np.float16)
    woutT = np.ascontiguousarray(np.asarray(w_out, dtype=np.float32).T.astype(np.float16))
    filtT = np.ascontiguousarray(np.asarray(spectral_filter, dtype=np.float32).T[:K, :])
    wqkT = np.ascontiguousarray(wqkvT[:, :QKW])
    wvT = np.ascontiguousarray(wqkvT[:, QKW:])
    in_maps = []
    for c in range(N_CORES):
        xh = x16[c].reshape(NT, TCH, D).transpose(1, 0, 2).reshape(TCH, NT * D)
        btp = basis16[c].T.reshape(K, 4, TQ).transpose(1, 0, 2).reshape(TCH, TQ)
        in_maps.append({
            "xH": np.ascontiguousarray(xh),
            "basis": np.ascontiguousarray(basis16[c]),
            "basisTp": np.ascontiguousarray(btp),
            "wqkT": wqkT,
            "wvT": wvT,
            "woutT": woutT,
            "filtT": filtT,
        })
    return in_maps


def _gather(res):
    out = np.empty((N_CORES, T, D), dtype=np.float32)
    for c in range(N_CORES):
        yt = res.results[c]["yT"].astype(np.float32)
        out[c] = yt.T * 128.0  # undo the on-device 2^-7 fold (exact)
    return out


def kernel(x, spectral_basis, w_qkv, w_out, spectral_filter):
    in_maps = _prep_in_maps(x, spectral_basis, w_qkv, w_out, spectral_filter)
    last_err = None
    for attempt in range(3):
        try:
            res = run_bass_kernel_spmd(_get_nc(), in_maps, core_ids=list(range(N_CORES)))
            break
        except Exception as e:  # transient NRT device errors recover on retry
            last_err = e
            import time
            time.sleep(2.0 * (attempt + 1))
    else:
        raise last_err
    return _gather(res)


def kernel_profiled(x, spectral_basis, w_qkv, w_out, spectral_filter, tmpdir=None):
    """Same as kernel() but with NTFF tracing; returns (out, BassKernelResults)."""
    in_maps = _prep_in_maps(x, spectral_basis, w_qkv, w_out, spectral_filter)
    res = run_bass_kernel_spmd(
        _get_nc(), in_maps, core_ids=list(range(N_CORES)),
        trace=True, trace_cores=list(range(N_CORES)), tmpdir=tmpdir,
    )
    return _gather(res), res


# revision 23
# speedup vs baseline: 1.2771x; 1.0819x over previous
"""FHN spectral attention kernel for 8 TRN2 NeuronCores.

Data-parallel over B=8 (one batch element per core). The reference math is
reassociated exactly so the [T,D]@[D,3D] qkv matmul never happens:

    xs[k,d]      = sum_t basis[t,k] x[t,d]                  (contract T)
    qkv_spec     = xs @ w_qkv.T          [32, 2304]
    attn[k,h]    = sum_d q_spec*k_spec / sqrt(64) * sigmoid(filt)
    fhn          = FHN(attn)             [32, 12]
    out_spec     = fhn (bcast d) * v_spec                    [32, 768]
    final_spec   = out_spec @ w_out.T    [32, 768]
    y.T[e,t]     = sum_k final_spec[k,e] basis[t,k]          (expand T)

All HBM-crossing tensors are f16 (pass gate is 2e-2 of absmax; measured error
~1e-3). y overflows f16 range, so 2^-7 is folded into the tiny fhn tile on
device and the host multiplies the output by 128 (exact).

Performance structure:
- x is host-packed so each partition's DMA line is one contiguous run.
- xs reduction col-packs 4 t-chunks into the PE array's 4 column groups
  (concurrent matmuls); the 4 partial sums are reduced + transposed in one
  matmul against a stacked identity.
- qkv weights are split q|k (streamed first) vs v (streamed last): the whole
  attention-scalar + FHN vector chain runs while the v/wout weights are still
  streaming in, taking it off the critical path.
- fhn is expanded to [d, k] layout by two tiny matmuls (a transpose against a
  repeated identity, then a head-selector matmul), so out_spec is built
  directly in the transposed layout the w_out projection needs -- no
  serial transpose pass after FHN.
- the y expansion row-packs the 4 t-quarters into the PE's 4 row groups
  (basisT loaded quarter-packed, final_spec replicated via the repeated
  identity), with [128,1024] PSUM tiles so PSUM->SBUF copies amortize.
- junk warm-up matmuls keep the PE HAM clock-gate at 8/8 through the DMA
  phases so the middle/expand matmuls run at 2.4 GHz, not 1.2.
"""

import numpy as np

import concourse.bass as bass
import concourse.mybir as mybir
from concourse import bacc
import concourse.tile as tile
from concourse.bass_utils import run_bass_kernel_spmd
from concourse.masks import make_identity

F32 = mybir.dt.float32
F16 = mybir.dt.float16

T, D = 4096, 768
H, HD, K = 12, 64, 32
D2 = 2 * D
D3 = 3 * D
QKW = 2 * D         # q|k weight columns
N_CORES = 8

TCH = 128           # t rows per matmul chunk
NT = T // TCH       # 32
XB = 4              # t-chunks per x DMA == col-pack width
DCH = 128           # d per chunk
ND = D // DCH       # 6
TQ = T // 4         # 1024, t per row-pack quarter

TAU, THRESH = 12.5, 0.5
A_PARAM, B_PARAM, DT = 0.7, 0.8, 1.0
ALPHA = DT / TAU
INV_DENOM = 1.0 / (1.0 + ALPHA * B_PARAM)
WC = ALPHA * INV_DENOM
OSHIFT = 2.0 ** -7  # fold into fhn so out_spec/final_spec/y all fit f16


def _mm_slices(total, step=512):
    out = []
    s = 0
    while s < total:
        out.append((s, min(step, total - s)))
        s += step
    return out


def build_nc() -> bass.Bass:
    nc = bacc.Bacc(None, target_bir_lowering=False)

    xH = nc.dram_tensor("xH", [TCH, NT * D], F16, kind="ExternalInput")
    basis = nc.dram_tensor("basis", [T, K], F16, kind="ExternalInput")
    basisTp = nc.dram_tensor("basisTp", [TCH, TQ], F16, kind="ExternalInput")
    wqkT = nc.dram_tensor("wqkT", [D, QKW], F16, kind="ExternalInput")
    wvT = nc.dram_tensor("wvT", [D, D], F16, kind="ExternalInput")
    woutT = nc.dram_tensor("woutT", [D, D], F16, kind="ExternalInput")
    filtT = nc.dram_tensor("filtT", [K, H], F32, kind="ExternalInput")
    yT = nc.dram_tensor("yT", [D, T], F16, kind="ExternalOutput")

    with tile.TileContext(nc) as tc:
        _body(tc, xH, basis, basisTp, wqkT, wvT, woutT, filtT, yT)
    nc.finalize()
    return nc


def _body(tc, xH, basis, basisTp, wqkT, wvT, woutT, filtT, yT):
    nc = tc.nc
    ActF = mybir.ActivationFunctionType
    Alu = mybir.AluOpType

    with (
        tc.tile_pool(name="singles", bufs=1) as singles,
        tc.tile_pool(name="xin", bufs=4) as xin,
        tc.tile_pool(name="spec", bufs=1) as spec,
        tc.tile_pool(name="fhn", bufs=1) as fp,
        tc.tile_pool(name="yout", bufs=3) as yout,
    ):
        # ---- constants (no DMA; engines are idle here anyway) --------------
        neg5 = singles.tile([K, 1], F32)
        nc.vector.memset(neg5, -5.0)
        # ident4[32g+k, k'] = d(k,k'): reduces 4 col-group partials via matmul
        ident4 = singles.tile([TCH, K], F16)
        nc.gpsimd.memset(ident4, 0.0)
        for g in range(4):
            make_identity(nc, ident4[32 * g:32 * (g + 1), :], nomemset=True)
        # ident_rep[k, 32g+k'] = d(k,k'): transpose that also replicates x4
        ident_rep = singles.tile([K, TCH], F16)
        nc.gpsimd.memset(ident_rep, 0.0)
        for g in range(4):
            make_identity(nc, ident_rep[:, 32 * g:32 * (g + 1)], nomemset=True)
        # S_all[h, d] = 1 iff h == d//64: head-selector for the fhn expansion
        S_all = singles.tile([H, D], F16)
        nc.gpsimd.memset(S_all, 0.0)
        sview = S_all.rearrange("h (d1 e) -> h d1 e", e=HD)
        nc.gpsimd.affine_select(
            out=sview, in_=sview, compare_op=mybir.AluOpType.not_equal,
            fill=1.0, base=0, pattern=[[-1, H], [0, HD]], channel_multiplier=1,
        )

        sb_filtT = singles.tile([K, H], F32)
        filt_sig = singles.tile([K, H], F32)
        sb_basis = singles.tile([TCH, NT, K], F16)
        bre = basis[:, :].rearrange("(n p) k -> p n k", p=TCH)

        sb_wqkT = singles.tile([DCH, ND, QKW], F16)
        sb_wvT = singles.tile([DCH, ND, D], F16)
        sb_woutT = singles.tile([DCH, ND, D], F16)
        sb_basisTp = singles.tile([TCH, TQ], F16)

        # ---- x phase: xs4[32g+k, d] = sum_{i%4==g} basis[chunk i] @ x ------
        xsT_f = spec.tile([DCH, ND, K], F16)
        with (
            tc.tile_pool(name="psA", bufs=1, space="PSUM") as psA,
            tc.tile_pool(name="psT", bufs=2, space="PSUM") as psT,
            tc.tile_pool(name="psJ", bufs=1, space="PSUM") as psJ,
        ):
            ps_xs4 = psA.tile([TCH, D], F32, tag="ps_xs4")
            groups = [(0, 1), (1, 3)]
            pos = XB
            while pos < NT:
                groups.append((pos, min(XB, NT - pos)))
                pos += XB
            nc.sync.dma_start(sb_basis[:, 0:8, :], bre[:, 0:8, :])
            for gidx, (gi, n) in enumerate(groups):
                x_tile = xin.tile([TCH, XB, D], F16, name="x_tile", tag="x_tile")
                src = xH[:, gi * D:(gi + n) * D].rearrange("p (n d) -> p n d", d=D)
                nc.sync.dma_start(x_tile[:, :n, :], src)
                if gidx == 0:
                    for bq in range(1, 4):
                        nc.sync.dma_start(sb_basis[:, bq * 8:(bq + 1) * 8, :],
                                          bre[:, bq * 8:(bq + 1) * 8, :])
                    nc.sync.dma_start(sb_filtT, filtT[:, :])
                    nc.scalar.activation(filt_sig, sb_filtT, ActF.Sigmoid)
                for j in range(n):
                    i = gi + j
                    g = i % 4
                    for (s, w) in _mm_slices(D):
                        nc.tensor.matmul(
                            ps_xs4[32 * g:32 * (g + 1), s:s + w],
                            lhsT=sb_basis[:, i, :],
                            rhs=x_tile[:, j, s:s + w],
                            start=(i < 4),
                            stop=(i >= NT - 4),
                            tile_position=(0, 32 * g),
                            skip_group_check=True,
                        )
                # HAM warm-up: keep the PE busy through the DMA wait so the
                # clock gate opens early and stays open (results unused)
                for _ in range(2):
                    ps_j = psJ.tile([K, 512], F32, tag="ps_j")
                    nc.tensor.matmul(
                        ps_j, lhsT=sb_basis[:, gi, :], rhs=x_tile[:, 0, 0:512],
                        start=True, stop=True,
                    )
            # weights stream behind the x DMAs, in consumption order.
            # wqkT is split 12 ways so the per-queue round-robin gives it
            # most of the bandwidth: it is consumed first but would otherwise
            # finish LAST (fair sharing drains small late transfers first).
            for dc in range(ND):
                for hf in range(2):
                    nc.sync.dma_start(
                        sb_wqkT[:, dc, hf * D:(hf + 1) * D],
                        wqkT[dc * DCH:(dc + 1) * DCH, hf * D:(hf + 1) * D])
            wvre = wvT[:, :].rearrange("(n p) d -> p n d", p=DCH)
            wore = woutT[:, :].rearrange("(n p) d -> p n d", p=DCH)
            nc.sync.dma_start(sb_wvT[:, 0:3, :], wvre[:, 0:3, :])
            nc.sync.dma_start(sb_wvT[:, 3:6, :], wvre[:, 3:6, :])
            nc.sync.dma_start(sb_woutT[:, 0:3, :], wore[:, 0:3, :])
            nc.sync.dma_start(sb_woutT[:, 3:6, :], wore[:, 3:6, :])
            nc.sync.dma_start(sb_basisTp, basisTp[:, :])

            # xs partials -> f16 SBUF; matmul vs stacked identity then does
            # the 4-way partial reduce AND the transpose in one shot
            sb_xs4 = spec.tile([TCH, D], F16)
            nc.vector.tensor_copy(sb_xs4[:, 0:D // 2], ps_xs4[:, 0:D // 2])
            nc.scalar.copy(sb_xs4[:, D // 2:D], ps_xs4[:, D // 2:D])
            for dc in range(ND):
                ps_t = psT.tile([DCH, K], F32, tag="ps_t")
                nc.tensor.matmul(
                    ps_t, lhsT=sb_xs4[:, dc * DCH:(dc + 1) * DCH], rhs=ident4,
                    start=True, stop=True,
                )
                if dc % 2 == 0:
                    nc.vector.tensor_copy(xsT_f[:, dc, :], ps_t)
                else:
                    nc.scalar.copy(xsT_f[:, dc, :], ps_t)

        # ---- spectral middle ------------------------------------------------
        sb_v = spec.tile([K, D], F16)
        vT4 = spec.tile([DCH, ND, TCH], F16)
        sb_osT4 = spec.tile([DCH, ND, TCH], F16)
        sb_fs4 = spec.tile([TCH, D], F16)
        with (
            tc.tile_pool(name="psQK", bufs=1, space="PSUM") as psQK,
            tc.tile_pool(name="psV", bufs=1, space="PSUM") as psV,
        ):
            ps_qk = psQK.tile([K, QKW], F32, tag="ps_qk")
            for dc in range(ND):
                for (s, w) in _mm_slices(QKW):
                    nc.tensor.matmul(
                        ps_qk[:, s:s + w],
                        lhsT=xsT_f[:, dc, :], rhs=sb_wqkT[:, dc, s:s + w],
                        start=(dc == 0), stop=(dc == ND - 1),
                    )
            ps_v = psV.tile([K, D], F32, tag="ps_v")
            for dc in range(ND):
                for (s, w) in _mm_slices(D):
                    nc.tensor.matmul(
                        ps_v[:, s:s + w],
                        lhsT=xsT_f[:, dc, :], rhs=sb_wvT[:, dc, s:s + w],
                        start=(dc == 0), stop=(dc == ND - 1),
                    )

            # attention scalar: attn[k,h] = sum_d q*k / 8 * sigmoid(filt)
            sb_q = fp.tile([K, D], F32)
            nc.vector.tensor_copy(sb_q[:, 0:D // 2], ps_qk[:, 0:D // 2])
            nc.scalar.copy(sb_q[:, D // 2:D], ps_qk[:, D // 2:D])
            prod = fp.tile([K, D], F32)
            nc.vector.tensor_tensor(prod, sb_q, ps_qk[:, D:QKW], op=Alu.mult)
            red = fp.tile([K, H], F32)
            nc.vector.reduce_sum(
                red, prod.rearrange("p (h d) -> p h d", d=HD), axis=mybir.AxisListType.X
            )
            # FHN, DVE chain with the sigmoid/affine pieces on ACT in parallel
            stim = fp.tile([K, H], F32)
            nc.vector.scalar_tensor_tensor(
                stim, red, 1.0 / (HD ** 0.5), filt_sig, op0=Alu.mult, op1=Alu.mult
            )
            ab = fp.tile([K, H], F32)
            nc.vector.scalar_tensor_tensor(
                ab, stim, -1.0, stim, op0=Alu.mult, op1=Alu.max
            )
            scale = fp.tile([K, H], F32)
            nc.vector.tensor_scalar_max(scale, ab, 1e-6)
            gate = fp.tile([K, H], F32)
            nc.scalar.activation(gate, scale, ActF.Sigmoid, bias=neg5, scale=10.0)
            g9 = fp.tile([K, H], F32)
            nc.scalar.activation(g9, gate, ActF.Copy, bias=0.1, scale=0.9)
            rscale = fp.tile([K, H], F32)
            nc.vector.reciprocal(rscale, scale)
            sn = fp.tile([K, H], F32)
            nc.vector.tensor_mul(sn, stim, rscale)
            v1 = fp.tile([K, H], F32)   # == I (first FHN step from v=w=0, |I|<=1)
            nc.vector.tensor_mul(v1, sn, g9)
            w1 = fp.tile([K, H], F32)   # (v1+A)*alpha/denom; clip never binds
            nc.scalar.activation(w1, v1, ActF.Copy, bias=A_PARAM * WC, scale=WC)
            # v2 = clip(3*v1 - v1^3/3 - w1, +-3)   (dv uses I == v1)
            c1 = fp.tile([K, H], F32)
            nc.vector.tensor_mul(c1, v1, v1)
            c2 = fp.tile([K, H], F32)
            nc.vector.tensor_mul(c2, c1, v1)
            u1 = fp.tile([K, H], F32)
            nc.vector.scalar_tensor_tensor(
                u1, c2, -1.0 / 3.0, w1, op0=Alu.mult, op1=Alu.subtract
            )
            u2 = fp.tile([K, H], F32)
            nc.vector.scalar_tensor_tensor(
                u2, v1, 3.0, u1, op0=Alu.mult, op1=Alu.add
            )
            v2 = fp.tile([K, H], F32)
            nc.vector.tensor_scalar(v2, u2, 3.0, -3.0, op0=Alu.min, op1=Alu.max)
            fhn16 = fp.tile([K, H], F16)  # v2 * scale * 2^-7 (host undoes 2^-7)
            nc.vector.scalar_tensor_tensor(
                fhn16, v2, OSHIFT, scale, op0=Alu.mult, op1=Alu.mult
            )

            # v_spec to SBUF on ACT only, after the FHN ACT pieces, so the DVE
            # chain never waits on the (late) v weight stream
            nc.scalar.copy(sb_v[:, 0:D // 2], ps_v[:, 0:D // 2])
            nc.scalar.copy(sb_v[:, D // 2:D], ps_v[:, D // 2:D])

        with (
            tc.tile_pool(name="psT2", bufs=2, space="PSUM") as psT2,
            tc.tile_pool(name="psE", bufs=2, space="PSUM") as psE,
            tc.tile_pool(name="psF", bufs=1, space="PSUM") as psF,
        ):
            # vT4[d, 32g+k] = v_spec[k, d] replicated into all 4 row groups
            for dc in range(ND):
                ps_t2 = psT2.tile([DCH, TCH], F32, tag="ps_t2")
                nc.tensor.matmul(
                    ps_t2, lhsT=sb_v[:, dc * DCH:(dc + 1) * DCH], rhs=ident_rep,
                    start=True, stop=True,
                )
                nc.scalar.copy(vT4[:, dc, :], ps_t2)
            # fhnT_rep[h, 32g+k] = fhn[k, h]
            ps_fT = psT2.tile([H, TCH], F32, tag="ps_fT")
            nc.tensor.matmul(ps_fT, lhsT=fhn16, rhs=ident_rep, start=True, stop=True)
            fhnT_sb = fp.tile([H, TCH], F16)
            nc.vector.tensor_copy(fhnT_sb, ps_fT)
            # per d-chunk: E[d, 32g+k] = fhn[k, h(d)] via the head selector,
            # then out_specT = v_specT * E, then the w_out projection
            ps_fs4 = psF.tile([TCH, D], F32, tag="ps_fs4")
            for dc in range(ND):
                ps_e = psE.tile([DCH, TCH], F32, tag="ps_e")
                nc.tensor.matmul(
                    ps_e, lhsT=S_all[:, dc * DCH:(dc + 1) * DCH], rhs=fhnT_sb,
                    start=True, stop=True,
                )
                nc.vector.tensor_tensor(
                    sb_osT4[:, dc, :], vT4[:, dc, :], ps_e, op=Alu.mult
                )
                for (s, w) in _mm_slices(D):
                    nc.tensor.matmul(
                        ps_fs4[:, s:s + w],
                        lhsT=sb_osT4[:, dc, :], rhs=sb_woutT[:, dc, s:s + w],
                        start=(dc == 0), stop=(dc == ND - 1),
                    )
            nc.vector.tensor_copy(sb_fs4[:, 0:D // 2], ps_fs4[:, 0:D // 2])
            nc.scalar.copy(sb_fs4[:, D // 2:D], ps_fs4[:, D // 2:D])

        # ---- yT[e, t] = sum_k final_spec[k, e] * basisT[k, t] ---------------
        # row-packed 4x: row group g computes t-quarter g; [128,1024] PSUM
        # tiles so each PSUM->SBUF copy moves 2 banks; one 1MB DMA per e-chunk
        with tc.tile_pool(name="psY", bufs=4, space="PSUM") as psY:
            for ec in range(ND):
                y_tile = yout.tile([DCH, T], F16, name="y_tile", tag="y_tile")
                for g in range(4):
                    ps_y = psY.tile([DCH, TQ], F32, tag="ps_y")
                    for (s, w) in _mm_slices(TQ):
                        nc.tensor.matmul(
                            ps_y[:, s:s + w],
                            lhsT=sb_fs4[32 * g:32 * (g + 1), ec * DCH:(ec + 1) * DCH],
                            rhs=sb_basisTp[32 * g:32 * (g + 1), s:s + w],
                            start=True, stop=True,
                            tile_position=(32 * g, 0),
                        )
                    dst = y_tile[:, g * TQ:(g + 1) * TQ]
                    if (ec + g) % 2 == 0:
                        nc.vector.tensor_copy(dst, ps_y)
                    else:
                        nc.scalar.copy(dst, ps_y)
                nc.sync.dma_start(yT[ec * DCH:(ec + 1) * DCH, :], y_tile)


_NC_CACHE = None


def _get_nc():
    global _NC_CACHE
    if _NC_CACHE is None:
        _NC_CACHE = build_nc()
    return _NC_CACHE


def _prep_in_maps(x, spectral_basis, w_qkv, w_out, spectral_filter):
    x16 = np.asarray(x, dtype=np.float16)
    basis16 = np.asarray(spectral_basis, dtype=np.float32).astype(np.float16)
    wqkvT = np.asarray(w_qkv, dtype=np.float32).T.astype(np.float16)
    woutT = np.ascontiguousarray(np.asarray(w_out, dtype=np.float32).T.astype(np.float16))
    filtT = np.ascontiguousarray(np.asarray(spectral_filter, dtype=np.float32).T[:K, :])
    wqkT = np.ascontiguousarray(wqkvT[:, :QKW])
    wvT = np.ascontiguousarray(wqkvT[:, QKW:])
    in_maps = []
    for c in range(N_CORES):
        xh = x16[c].reshape(NT, TCH, D).transpose(1, 0, 2).reshape(TCH, NT * D)
        btp = basis16[c].T.reshape(K, 4, TQ).transpose(1, 0, 2).reshape(TCH, TQ)
        in_maps.append({
            "xH": np.ascontiguousarray(xh),
            "basis": np.ascontiguousarray(basis16[c]),
            "basisTp": np.ascontiguousarray(btp),
            "wqkT": wqkT,
            "wvT": wvT,
            "woutT": woutT,
            "filtT": filtT,
        })
    return in_maps


def _gather(res):
    out = np.empty((N_CORES, T, D), dtype=np.float32)
    for c in range(N_CORES):
        yt = res.results[c]["yT"].astype(np.float32)
        out[c] = yt.T * 128.0  # undo the on-device 2^-7 fold (exact)
    return out


def kernel(x, spectral_basis, w_qkv, w_out, spectral_filter):
    in_maps = _prep_in_maps(x, spectral_basis, w_qkv, w_out, spectral_filter)
    last_err = None
    for attempt in range(3):
        try:
            res = run_bass_kernel_spmd(_get_nc(), in_maps, core_ids=list(range(N_CORES)))
            break
        except Exception as e:  # transient NRT device errors recover on retry
            last_err = e
            import time
            time.sleep(2.0 * (attempt + 1))
    else:
        raise last_err
    return _gather(res)


def kernel_profiled(x, spectral_basis, w_qkv, w_out, spectral_filter, tmpdir=None):
    """Same as kernel() but with NTFF tracing; returns (out, BassKernelResults)."""
    in_maps = _prep_in_maps(x, spectral_basis, w_qkv, w_out, spectral_filter)
    res = run_bass_kernel_spmd(
        _get_nc(), in_maps, core_ids=list(range(N_CORES)),
        trace=True, trace_cores=list(range(N_CORES)), tmpdir=tmpdir,
    )
    return _gather(res), res
